# revision 8
# baseline (speedup 1.0000x reference)
"""FBSNN loss kernel for Trainium2, data-parallel over Monte-Carlo paths.

Strategy (8 NeuronCores, paths sharded 512/core, MLP weights replicated):
  * All 51 time-step MLP evaluations are independent once the state path
    X_n = X_{n-1} + SIG*X_{n-1}*dW_{n-1} is known, and X's recurrence does
    not involve the network.  Per step the kernel runs the 5-layer MLP
    forward (fp32 matmuls) + the VJP backward (bf16 matmuls - the loss is
    insensitive to backward precision) on all 512 paths, feature-major
    (features on partitions, paths on the free axis), then accumulates the
    Euler residual terms.  Each core emits one partial loss scalar; the
    host sums the 8 partials and divides by M.
  * sin/cos run on the scalar engine's Sin table, which is only valid on
    ~[-pi,pi]; arguments are range-reduced with a magic-number
    round-to-nearest-multiple-of-2pi on the vector engine, and
    cos(y)=sin(pi/2-|y|) reuses the same reduced argument.
  * The host pre-transposes t / dW to feature-major and pre-chunks the
    weights so the device does no transposes at all.
"""

import numpy as np
import ml_dtypes

import concourse.bass as bass
import concourse.tile as tile
from concourse import bacc, mybir
from concourse import bass_utils
from concourse.bass import ts

N_CORES = 8
M, NT, D, HID = 4096, 51, 100, 512     # paths, time points, state dim, hidden
PPC = M // N_CORES                     # paths per core = 512
R, SIG = 0.05, 0.4
KC_ = HID // 128                       # 4 contraction chunks of 128

PI = float(np.pi)
TWO_PI_F = float(np.float32(2 * np.pi))
INV2PI = float(np.float32(1.0 / (2 * np.pi)))
MAGIC = float(np.float32(1.5 * 2 ** 23))
HALF_PI = float(np.pi / 2)

F32 = mybir.dt.float32
BF16 = mybir.dt.bfloat16
AOP = mybir.AluOpType
SIN = mybir.ActivationFunctionType.Sin


def build_program():
    nc = bacc.Bacc()

    # ---- DRAM I/O (per core) ----
    dWT_d = nc.dram_tensor("dWT", [NT - 1, D, PPC], F32, kind="ExternalInput")
    tT_d = nc.dram_tensor("tT", [NT, PPC], F32, kind="ExternalInput")
    dtT_d = nc.dram_tensor("dtT", [NT - 1, PPC], F32, kind="ExternalInput")
    X0T_d = nc.dram_tensor("X0T", [D, PPC], F32, kind="ExternalInput")
    W0_d = nc.dram_tensor("W0", [D + 1, HID], F32, kind="ExternalInput")
    Wd = {l: nc.dram_tensor(f"W{l}", [HID, HID], F32, kind="ExternalInput")
          for l in (1, 2, 3)}
    W4c_d = nc.dram_tensor("W4c", [128, KC_], F32, kind="ExternalInput")
    WTd = {l: nc.dram_tensor(f"W{l}T", [HID, HID], BF16, kind="ExternalInput")
           for l in (1, 2, 3)}
    W0T_d = nc.dram_tensor("W0T", [HID, D], BF16, kind="ExternalInput")
    biasA_d = {l: nc.dram_tensor(f"biasA{l}", [HID], F32, kind="ExternalInput")
               for l in range(4)}
    b_d = {l: nc.dram_tensor(f"b{l}", [HID], F32, kind="ExternalInput")
           for l in range(4)}
    b4_d = nc.dram_tensor("b4", [1], F32, kind="ExternalInput")
    loss_d = nc.dram_tensor("loss", [1, 1], F32, kind="ExternalOutput")

    with tile.TileContext(nc) as tc:
        with (
            tc.tile_pool(name="const", bufs=1) as const,
            tc.tile_pool(name="xp", bufs=3) as xp,
            tc.tile_pool(name="dwp", bufs=3) as dwp,
            tc.tile_pool(name="hp", bufs=2) as hp,
            tc.tile_pool(name="ap", bufs=1) as apl,
            tc.tile_pool(name="yp", bufs=2) as yp,
            tc.tile_pool(name="cp", bufs=5) as cp,
            tc.tile_pool(name="gzp", bufs=3) as gzp,
            tc.tile_pool(name="zp", bufs=2) as zp,
            tc.tile_pool(name="rt", bufs=2) as rt,
            tc.tile_pool(name="stg", bufs=3) as stg,
            tc.tile_pool(name="accp", bufs=1) as accp,
            tc.tile_pool(name="ftp", bufs=2) as ftp,
            tc.tile_pool(name="zg", bufs=5, space="PSUM") as zg,
            tc.tile_pool(name="sm", bufs=3, space="PSUM") as sm,
        ):
            # ---- load constants / weights ----
            W0sb = const.tile([D + 1, HID], F32)
            nc.sync.dma_start(out=W0sb, in_=W0_d[:, :])
            Wsb = {}
            for l in (1, 2, 3):
                Wsb[l] = const.tile([128, KC_, HID], F32, name=f"Wsb{l}")
                nc.sync.dma_start(out=Wsb[l],
                                  in_=Wd[l].rearrange("(k p) n -> p k n", p=128))
            W4sb = const.tile([128, KC_], F32)
            nc.sync.dma_start(out=W4sb, in_=W4c_d[:, :])
            WTsb = {}
            for l in (1, 2, 3):
                WTsb[l] = const.tile([128, KC_, HID], BF16, name=f"WTsb{l}")
                nc.sync.dma_start(out=WTsb[l],
                                  in_=WTd[l].rearrange("(k p) n -> p k n", p=128))
            W0Tsb = const.tile([128, KC_, D], BF16)
            nc.sync.dma_start(out=W0Tsb,
                              in_=W0T_d.rearrange("(k p) m -> p k m", p=128))
            biasAsb, bsb = {}, {}
            for l in range(4):
                biasAsb[l] = const.tile([128, KC_], F32, name=f"biasAsb{l}")
                nc.sync.dma_start(out=biasAsb[l],
                                  in_=biasA_d[l].rearrange("(m p) -> p m", p=128))
                bsb[l] = const.tile([128, KC_], F32, name=f"bsb{l}")
                nc.sync.dma_start(out=bsb[l],
                                  in_=b_d[l].rearrange("(m p) -> p m", p=128))
            b4sb = const.tile([1, 1], F32)
            nc.sync.dma_start(out=b4sb, in_=b4_d[None, :])
            dtsb = const.tile([NT - 1, PPC], F32)
            nc.sync.dma_start(out=dtsb, in_=dtT_d[:, :])
            ones = const.tile([D, 1], F32)
            nc.vector.memset(ones, 1.0)
            hpi = const.tile([128, 1], F32)
            nc.vector.memset(hpi, HALF_PI)
            # accumulators: Ya[n]=Y_n (n<=49), Yb[n]=Y_{n+1}, S1a[n], S2a[n]
            Ya = accp.tile([NT - 1, PPC], F32)
            Yb = accp.tile([NT - 1, PPC], F32)
            S1a = accp.tile([NT - 1, PPC], F32)
            S2a = accp.tile([NT - 1, PPC], F32)

            # ---- initial state tile: rows 0..99 = X_0^T, row 100 = t_0 ----
            # (W0 is host-permuted to the same row order)
            x_cur = xp.tile([D + 1, PPC], F32, tag="x")
            nc.sync.dma_start(out=x_cur[0:D, :], in_=X0T_d[:, :])
            nc.sync.dma_start(out=x_cur[D:D + 1, :], in_=tT_d[0:1, :])

            # prefetch first dWT slices
            dw_tiles = {}
            PREFETCH = 3
            for n in range(min(PREFETCH, NT - 1)):
                dw_tiles[n] = dwp.tile([D, PPC], F32, name=f"dwt{n}", tag="dwt")
                nc.sync.dma_start(out=dw_tiles[n], in_=dWT_d[n])

            ystage_last = None
            Zt_last = None
            x_last = None

            for n in range(NT):
                # ---------- X recurrence (produces x_{n+1}; cheap, runs ahead)
                sdW = None
                if n < NT - 1:
                    if n + PREFETCH < NT - 1:
                        m_ = n + PREFETCH
                        dw_tiles[m_] = dwp.tile([D, PPC], F32, name=f"dwt{m_}", tag="dwt")
                        nc.sync.dma_start(out=dw_tiles[m_], in_=dWT_d[m_])
                    aX = rt.tile([D, PPC], F32, tag="aX")
                    nc.vector.tensor_scalar(aX, x_cur[0:D, :], SIG, None,
                                            AOP.mult)
                    sdW = rt.tile([D, PPC], F32, tag="sdW")
                    nc.vector.tensor_tensor(sdW, aX, dw_tiles[n], AOP.mult)
                    x_next = xp.tile([D + 1, PPC], F32, tag="x")
                    nc.vector.tensor_tensor(x_next[0:D, :], x_cur[0:D, :],
                                            sdW, AOP.add)
                    nc.sync.dma_start(out=x_next[D:D + 1, :], in_=tT_d[n + 1:n + 2, :])
                    del dw_tiles[n]
                else:
                    x_next = None

                # ---------- forward MLP ----------
                h_prev = None
                cs = []
                for l in range(4):
                    h_new = hp.tile([128, KC_, PPC], F32)
                    ytile = yp.tile([128, KC_, PPC], F32)
                    Atile = apl.tile([128, KC_, PPC], F32)
                    for m in range(KC_):
                        z = zg.tile([128, PPC], F32, tag="zg")
                        if l == 0:
                            nc.tensor.matmul(z, W0sb[:, ts(m, 128)], x_cur[:, :],
                                             start=True, stop=True)
                        else:
                            for k in range(KC_):
                                nc.tensor.matmul(z, Wsb[l][:, k, ts(m, 128)],
                                                 h_prev[:, k, :],
                                                 start=(k == 0), stop=(k == KC_ - 1))
                        # A = z*inv2pi + (b*inv2pi + magic)
                        nc.vector.tensor_scalar(Atile[:, m, :], z, INV2PI,
                                                biasAsb[l][:, m:m + 1],
                                                AOP.mult, AOP.add)
                        # KC = (A - magic) * 2pi   (in place)
                        nc.vector.tensor_scalar(Atile[:, m, :], Atile[:, m, :],
                                                MAGIC, TWO_PI_F,
                                                AOP.subtract, AOP.mult)
                        # y = (z + b) - KC
                        nc.vector.scalar_tensor_tensor(
                            ytile[:, m, :], z, bsb[l][:, m:m + 1], Atile[:, m, :],
                            AOP.add, AOP.subtract)
                        # h = sin(y)
                        nc.scalar.activation(out=h_new[:, m, :], in_=ytile[:, m, :],
                                             func=SIN)
                    # |y| in place, then cos = sin(pi/2 - |y|)  (bf16, bwd only)
                    nc.vector.scalar_tensor_tensor(
                        ytile[:, :, :], ytile[:, :, :], -1.0, ytile[:, :, :],
                        AOP.mult, AOP.max)
                    c_l = cp.tile([128, KC_, PPC], BF16)
                    nc.scalar.activation(out=c_l[:, :, :], in_=ytile[:, :, :],
                                         func=SIN, bias=hpi[:, 0:1], scale=-1.0)
                    cs.append(c_l)
                    h_prev = h_new

                # ---------- u = h4 @ W4 + b4 ----------
                z5 = sm.tile([1, PPC], F32, tag="sm")
                for k in range(KC_):
                    nc.tensor.matmul(z5, W4sb[:, k:k + 1], h_prev[:, k, :],
                                     start=(k == 0), stop=(k == KC_ - 1))
                ystage = stg.tile([1, PPC], F32, tag="ystage")
                nc.vector.tensor_scalar(ystage, z5, b4sb[0:1, 0:1], None, AOP.add)
                if n <= NT - 2:
                    nc.sync.dma_start(out=Ya[n:n + 1, :], in_=ystage)
                if n >= 1:
                    nc.sync.dma_start(out=Yb[n - 1:n, :], in_=ystage)

                # ---------- backward (bf16): Du w.r.t. X ----------
                gz = gzp.tile([128, KC_, PPC], BF16, tag="gz")
                for m in range(KC_):
                    nc.vector.tensor_scalar(gz[:, m, :], cs[3][:, m, :],
                                            W4sb[:, m:m + 1], None, AOP.mult)
                for l in (3, 2, 1):
                    gz_new = gzp.tile([128, KC_, PPC], BF16, tag="gz")
                    for m in range(KC_):
                        g = zg.tile([128, PPC], F32, tag="zg")
                        for k in range(KC_):
                            nc.tensor.matmul(g, WTsb[l][:, k, ts(m, 128)],
                                             gz[:, k, :],
                                             start=(k == 0), stop=(k == KC_ - 1))
                        nc.vector.tensor_tensor(gz_new[:, m, :], g,
                                                cs[l - 1][:, m, :], AOP.mult)
                    gz = gz_new
                g0 = zg.tile([D, PPC], F32, tag="zg")
                for k in range(KC_):
                    nc.tensor.matmul(g0, W0Tsb[:, k, :], gz[:, k, :],
                                     start=(k == 0), stop=(k == KC_ - 1))
                Zt = zp.tile([D, PPC], F32)
                nc.vector.tensor_copy(Zt, g0[:, :])

                # ---------- per-step residual terms ----------
                if n <= NT - 2:
                    XZ = rt.tile([D, PPC], F32, tag="prod")
                    nc.vector.tensor_tensor(XZ, x_cur[0:D, :], Zt, AOP.mult)
                    S1 = sm.tile([1, PPC], F32, tag="sm")
                    nc.tensor.matmul(S1, ones, XZ, start=True, stop=True)
                    ZsW = rt.tile([D, PPC], F32, tag="prod")
                    nc.vector.tensor_tensor(ZsW, Zt, sdW, AOP.mult)
                    S2 = sm.tile([1, PPC], F32, tag="sm")
                    nc.tensor.matmul(S2, ones, ZsW, start=True, stop=True)
                    s1s = stg.tile([1, PPC], F32, tag="sstg")
                    nc.vector.tensor_copy(s1s, S1)
                    nc.sync.dma_start(out=S1a[n:n + 1, :], in_=s1s)
                    s2s = stg.tile([1, PPC], F32, tag="sstg")
                    nc.vector.tensor_copy(s2s, S2)
                    nc.sync.dma_start(out=S2a[n:n + 1, :], in_=s2s)

                if n == NT - 1:
                    ystage_last = ystage
                    Zt_last = Zt
                    x_last = x_cur
                x_cur = x_next

            # ---------- terminal terms ----------
            XX = rt.tile([D, PPC], F32, tag="fin")
            nc.vector.tensor_tensor(XX, x_last[0:D, :], x_last[0:D, :],
                                    AOP.mult)
            gX = sm.tile([1, PPC], F32, tag="sm")
            nc.tensor.matmul(gX, ones, XX, start=True, stop=True)
            tU = stg.tile([1, PPC], F32, tag="finv")
            nc.vector.tensor_tensor(tU, ystage_last, gX, AOP.subtract)
            tU2 = stg.tile([1, PPC], F32, tag="finv")
            nc.vector.tensor_tensor(tU2, tU, tU, AOP.mult)
            D2 = rt.tile([D, PPC], F32, tag="fin")
            nc.vector.tensor_scalar(D2, x_last[0:D, :], 2.0, None, AOP.mult)
            DD = rt.tile([D, PPC], F32, tag="fin")
            nc.vector.tensor_tensor(DD, Zt_last, D2, AOP.subtract)
            DD2 = rt.tile([D, PPC], F32, tag="fin")
            nc.vector.tensor_tensor(DD2, DD, DD, AOP.mult)
            sD = sm.tile([1, PPC], F32, tag="sm")
            nc.tensor.matmul(sD, ones, DD2, start=True, stop=True)
            term = stg.tile([1, PPC], F32, tag="finv")
            nc.vector.tensor_tensor(term, tU2, sD, AOP.add)

            # ---------- batched residual + final reduction ----------
            rA = ftp.tile([NT - 1, PPC], F32, tag="ft")
            nc.vector.tensor_tensor(rA, Ya, S1a, AOP.subtract)
            ph = ftp.tile([NT - 1, PPC], F32, tag="ft")
            nc.vector.tensor_scalar(ph, rA, R, None, AOP.mult)
            pd = ftp.tile([NT - 1, PPC], F32, tag="ft")
            nc.vector.tensor_tensor(pd, ph, dtsb, AOP.mult)
            yt = ftp.tile([NT - 1, PPC], F32, tag="ft")
            nc.vector.tensor_tensor(yt, Ya, pd, AOP.add)
            yt2 = ftp.tile([NT - 1, PPC], F32, tag="ft")
            nc.vector.tensor_tensor(yt2, yt, S2a, AOP.add)
            resid = ftp.tile([NT - 1, PPC], F32, tag="ft")
            nc.vector.tensor_tensor(resid, Yb, yt2, AOP.subtract)
            sq = ftp.tile([NT - 1, PPC], F32, tag="ft")
            nc.vector.tensor_tensor(sq, resid, resid, AOP.mult)
            rs = ftp.tile([NT - 1, 1], F32, tag="rs")
            nc.vector.tensor_reduce(rs, sq, mybir.AxisListType.X, AOP.add)
            tot = sm.tile([1, 1], F32, tag="sm")
            nc.tensor.matmul(tot, ones[0:NT - 1, :], rs, start=True, stop=True)
            tsum = stg.tile([1, 1], F32, tag="sc")
            nc.vector.tensor_reduce(tsum, term, mybir.AxisListType.X, AOP.add)
            lsb = stg.tile([1, 1], F32, tag="sc")
            nc.vector.tensor_tensor(lsb, tsum, tot, AOP.add)
            nc.sync.dma_start(out=loss_d[:, :], in_=lsb)

    nc.compile()
    return nc


def prep_inputs(t, W, Xi, Wm0, b0, Wm1, b1, Wm2, b2, Wm3, b3, Wm4, b4):
    """Host-side sharding + layout prep. Returns in_maps for the 8 cores."""
    f32 = np.float32
    t = np.asarray(t, f32)
    W = np.asarray(W, f32)
    Xi = np.asarray(Xi, f32)
    tT_full = np.ascontiguousarray(t[:, :, 0].T)               # (51, 4096)
    dtT_full = tT_full[1:] - tT_full[:-1]                      # (50, 4096)
    WT_t = W.transpose(1, 2, 0)                                # (51, 100, 4096)
    dWT_full = WT_t[1:] - WT_t[:-1]                            # (50, 100, 4096)
    X0T = np.ascontiguousarray(
        np.broadcast_to(Xi.reshape(D, 1), (D, PPC))).astype(f32)

    Wm0 = np.asarray(Wm0, f32)
    W0perm = np.ascontiguousarray(np.concatenate([Wm0[1:], Wm0[0:1]], axis=0))
    Wm4 = np.asarray(Wm4, f32)
    W4c = np.ascontiguousarray(Wm4.reshape(KC_, 128).T)        # (128, 4)
    bf = ml_dtypes.bfloat16
    shared = {
        "X0T": X0T,
        "W0": W0perm,
        "W1": np.asarray(Wm1, f32), "W2": np.asarray(Wm2, f32),
        "W3": np.asarray(Wm3, f32),
        "W4c": W4c,
        "W1T": np.ascontiguousarray(np.asarray(Wm1, f32).T).astype(bf),
        "W2T": np.ascontiguousarray(np.asarray(Wm2, f32).T).astype(bf),
        "W3T": np.ascontiguousarray(np.asarray(Wm3, f32).T).astype(bf),
        "W0T": np.ascontiguousarray(Wm0[1:, :].T).astype(bf),
        "b4": np.asarray(b4, f32).reshape(1),
    }
    for l, bl in enumerate([b0, b1, b2, b3]):
        bl = np.asarray(bl, f32)
        shared[f"b{l}"] = bl
        shared[f"biasA{l}"] = (bl * f32(INV2PI) + f32(MAGIC)).astype(f32)

    in_maps = []
    for c in range(N_CORES):
        s = slice(c * PPC, (c + 1) * PPC)
        in_maps.append({
            **shared,
            "dWT": np.ascontiguousarray(dWT_full[:, :, s]),
            "tT": np.ascontiguousarray(tT_full[:, s]),
            "dtT": np.ascontiguousarray(dtT_full[:, s]),
        })
    return in_maps


_NC_CACHE = []


def run_device(in_maps, trace=False):
    if not _NC_CACHE:
        _NC_CACHE.append(build_program())
    nc = _NC_CACHE[0]
    res = bass_utils.run_bass_kernel_spmd(nc, in_maps,
                                          core_ids=list(range(N_CORES)),
                                          trace=trace)
    partials = [res.results[c]["loss"][0, 0] for c in range(N_CORES)]
    return partials, res


def kernel(**inputs) -> np.ndarray:
    in_maps = prep_inputs(**inputs)
    partials, _ = run_device(in_maps)
    total = np.float32(0.0)
    for p in partials:
        total = np.float32(total + np.float32(p))
    return np.asarray(np.float32(total / np.float32(M)))
